# revision 11
# baseline (speedup 1.0000x reference)
"""GAT encoder (gnn_message_passing) on 8 trn2 NeuronCores via Bass.

Strategy (graph-parallel, dst-sharded), v2:
  Phase 1 (node-sharded): one fp16 matmul chain per 128-node tile against
    [W1 | W1@att_src | W1@att_dst] -> rows [h(128) | es | ed] fp16.
  Host: build gather table htab (h rows, 256B fp16), route edges to
    dst-owner cores into windows of 128 dst nodes sorted by (low-degree,
    high-degree); per-slot es+ed bias; per-window pad counts.
  Phase 2 (per core): dma_gather bulk-fetches edge-slot h rows.  int16
    indices limit a gather table to 32768 rows, so each window has
    nchA low-src columns + nchB high-src columns, fetched by two calls
    per group from the two table halves (pad slots read a zero dummy row;
    every index is valid - HW emits descriptors for the first
    num_idxs_reg entries and skips nothing).  Attention without act-table
    swaps: exp(sigmoid(z)) = e^0.5 * exp(tanh(z/2)/2); the constant
    cancels in the softmax (pad slots use es=-60000 -> e^-0.5, removed
    from the denominator via pcw).  ex applied via one DVE fp16 multiply
    per pass per group; per-window PSUM accumulation via fp16 identity
    matmuls; ELU via max(x,0)+exp(min(x,0))-1 with the -1 folded into
    the output bias; y produced transposed and fixed up on host.
"""
import os
import sys
import time

sys.path.insert(0, "/opt/trn_rl_repo")

import numpy as np

N, E = 50000, 800000
IN, HID, OUT = 256, 128, 128
NCORES = 8
NPC = N // NCORES            # nodes per core (6250)
NW = (NPC + 127) // 128      # 49 windows; last partial (6250 = 48*128+106)
NPAD = NW * 128              # 6272
GCOLS = int(os.environ.get("BASS_GAT_GCOLS", "80"))  # max gather cols/group
CALLCOLS = int(os.environ.get("BASS_GAT_CALLCOLS", "64"))
SINGLE_PACKET = os.environ.get("BASS_GAT_SP", "0") == "1"

TABROWS = N + 3              # row 0 = zero (low dummy), 1..N = node src+1,
LOWROWS = 32768              # row N+2 = zero (high dummy)
HIBASE = 32767
DUMMY_HI_LOCAL = N + 2 - HIBASE

_timings = {}


def _patch_env():
    """Tile/perfetto compatibility patches for this container."""
    import concourse.tile as tile
    from concourse.tile import ScopedClock

    def _drain_and_barrier_split(self, tick_clock, wait_clock):
        nc = self.nc
        probe = nc.sync.nop()
        wait_clock.add_sem_waits(
            probe.ins, ScopedClock({None: tick_clock.global_clock})
        )
        waits = list(probe.ins.sync_info.on_wait or [])
        probe.ins.sync_info.on_wait = []
        from concourse import mybir

        for w in waits:
            inst = nc.sync.nop()
            if inst.ins.sync_info is None:
                inst.ins.sync_info = mybir.SyncInfo(on_wait=[w], on_update=[])
            else:
                inst.ins.sync_info.on_wait = [w]
        nc.sync.drain()
        nc.all_engine_barrier()
        assert self.sems is not None
        popped = nc._tile_sem_poison_stack.pop()
        assert popped is self._sem_poison
        nc.clear_and_free_semaphores(list(self.sems.allocated().values()))
        nc.all_engine_barrier()

    tile.TileContext._drain_and_barrier = _drain_and_barrier_split


_patch_env()


def _patch_perfetto():
    try:
        from gauge import trn_perfetto

        cls = trn_perfetto.TrnPerfettoConv
        if not getattr(cls, "_no_hlo_patched", False):
            _orig_init = cls.__init__

            def _init_no_hlo(self, *a, **k):
                k["annotate_hlo"] = False
                if len(a) >= 2:
                    a = (a[0], False) + a[2:]
                _orig_init(self, *a, **k)

            cls.__init__ = _init_no_hlo
            cls._no_hlo_patched = True
    except Exception:
        pass


import concourse.bass as bass
import concourse.bacc as bacc
import concourse.tile as tile
from concourse import mybir
from concourse.bass_utils import run_bass_kernel_spmd
from concourse.masks import make_identity

F32 = mybir.dt.float32
F16 = mybir.dt.float16
I16 = mybir.dt.int16
AF = mybir.ActivationFunctionType
ALU = mybir.AluOpType


# ---------------------------------------------------------------- phase 1
def build_phase1():
    """h/es/ed for this core's nodes: one fp16 matmul chain per tile."""
    nc = bacc.Bacc("TRN2", target_bir_lowering=True)
    xT = nc.dram_tensor("xT", [IN, NPAD], F16, kind="ExternalInput")
    wcat = nc.dram_tensor("wcat", [IN, HID + 2], F16, kind="ExternalInput")
    haug = nc.dram_tensor("haug", [NPAD, HID + 2], F16, kind="ExternalOutput")

    RF = HID + 2  # 130
    TB = 4        # tiles per DMA batch

    with tile.TileContext(nc) as tc:
        with (
            tc.tile_pool(name="sbuf", bufs=3) as pool,
            tc.tile_pool(name="cpool", bufs=1) as cpool,
            tc.tile_pool(name="psum", bufs=4, space="PSUM") as psum,
        ):
            w_t = cpool.tile([128, IN // 128, RF], F16)
            nc.sync.dma_start(
                out=w_t[:], in_=wcat[:].rearrange("(a k) f -> k a f", k=128)
            )
            nb = (NW + TB - 1) // TB
            for b in range(nb):
                t0 = b * TB
                tn = min(TB, NW - t0)
                xt = pool.tile([128, IN // 128, TB * 128], F16, tag="xt")
                nc.sync.dma_start(
                    out=xt[:, :, : tn * 128],
                    in_=xT[:, t0 * 128 : (t0 + tn) * 128].rearrange(
                        "(a k) n -> k a n", k=128
                    ),
                )
                ha = pool.tile([128, TB, RF], F16, tag="ha")
                for t in range(tn):
                    hp = psum.tile([128, RF], F32, tag="hp")
                    for a in range(IN // 128):
                        nc.tensor.matmul(
                            out=hp[:],
                            lhsT=xt[:, a, t * 128 : (t + 1) * 128],
                            rhs=w_t[:, a],
                            start=(a == 0),
                            stop=(a == IN // 128 - 1),
                        )
                    nc.scalar.activation(ha[:, t], hp[:], AF.Copy)
                nc.sync.dma_start(
                    out=haug[t0 * 128 : (t0 + tn) * 128, :].rearrange(
                        "(t k) f -> k t f", k=128
                    ),
                    in_=ha[:, :tn],
                )
    nc.finalize()
    return nc


# ---------------------------------------------------------------- phase 2
def build_phase2(nchA, nchB, groups):
    """Per-window low/high chunk counts and group spans."""
    NWl = len(nchA)
    offsA = np.zeros(NWl + 1, dtype=int)
    offsA[1:] = np.cumsum(nchA)
    offsB = np.zeros(NWl + 1, dtype=int)
    offsB[1:] = np.cumsum(nchB)
    TOTA, TOTB = int(offsA[-1]), int(offsB[-1])
    # idx segment start per group (8 wrapped cols per gather col, A then B)
    ico = np.zeros(len(groups) + 1, dtype=int)
    for g, (w0, w1_) in enumerate(groups):
        na = int(offsA[w1_] - offsA[w0])
        nb_ = int(offsB[w1_] - offsB[w0])
        ico[g + 1] = ico[g] + 8 * (na + nb_)
    ITOT = int(ico[-1])

    nc = bacc.Bacc("TRN2", target_bir_lowering=True, num_swdge_queues=4)
    htab = nc.dram_tensor("htab", [TABROWS, HID], F16, kind="ExternalInput")
    idxs = nc.dram_tensor("idxs", [128, ITOT], I16, kind="ExternalInput")
    esw = nc.dram_tensor("esw", [128, TOTA + TOTB], F16, kind="ExternalInput")
    pcw = nc.dram_tensor("pcw", [128, NW], F32, kind="ExternalInput")
    w2 = nc.dram_tensor("w2", [HID, OUT], F16, kind="ExternalInput")
    c2n = nc.dram_tensor("c2n", [OUT, 1], F32, kind="ExternalInput")
    yT = nc.dram_tensor("yT", [OUT, NW * 128], F16, kind="ExternalOutput")

    with tile.TileContext(nc) as tc:
        with (
            tc.tile_pool(name="gpool", bufs=4) as gpool,
            tc.tile_pool(name="mpool", bufs=3) as mpool,
            tc.tile_pool(name="spool", bufs=6) as spool,
            tc.tile_pool(name="hpool", bufs=3) as hpool,
            tc.tile_pool(name="cpool", bufs=1) as cpool,
            tc.tile_pool(name="psum", bufs=4, space="PSUM") as psum,
            tc.tile_pool(name="psumt", bufs=2, space="PSUM") as psumt,
            tc.tile_pool(name="psumy", bufs=2, space="PSUM") as psumy,
        ):
            ident = cpool.tile([128, 128], F16)
            make_identity(nc, ident[:])
            w2_t = cpool.tile([HID, OUT], F16)
            nc.sync.dma_start(out=w2_t[:], in_=w2[:])
            c2n_t = cpool.tile([OUT, 1], F32)
            nc.sync.dma_start(out=c2n_t[:], in_=c2n[:])
            pcw_t = cpool.tile([128, NW], F32)
            nc.sync.dma_start(out=pcw_t[:], in_=pcw[:])
            it_t = cpool.tile([128, ITOT], I16)
            nc.sync.dma_start(out=it_t[:], in_=idxs[:])
            qrr = [0]
            esw_t = cpool.tile([128, TOTA + TOTB], F16)
            nc.sync.dma_start(out=esw_t[:], in_=esw[:])

            # largest groups first: the pipeline drains on a small one
            for g, (w0, w1_) in reversed(list(enumerate(groups))):
                a0, a1 = int(offsA[w0]), int(offsA[w1_])
                b0, b1 = int(offsB[w0]), int(offsB[w1_])
                na, nb_ = a1 - a0, b1 - b0
                i0 = int(ico[g])

                gtA = gpool.tile([128, na * HID], F16, tag="gtA")
                gA3 = gtA[:].rearrange("p (c f) -> p c f", f=HID)
                for s0 in range(0, na, CALLCOLS):
                    sn = min(CALLCOLS, na - s0)
                    nc.gpsimd.dma_gather(
                        out_ap=gA3[:, s0 : s0 + sn],
                        in_ap=htab[:LOWROWS],
                        idxs_ap=it_t[:, i0 + 8 * s0 : i0 + 8 * (s0 + sn)],
                        num_idxs=128 * sn,
                        num_idxs_reg=128 * sn,
                        elem_size=HID,
                        single_packet=SINGLE_PACKET,
                        queue_num=qrr[0] % 4,
                    )
                    qrr[0] += 1
                gtB = gpool.tile([128, max(nb_, 1) * HID], F16, tag="gtB")
                gB3 = gtB[:].rearrange("p (c f) -> p c f", f=HID)
                for s0 in range(0, nb_, CALLCOLS):
                    sn = min(CALLCOLS, nb_ - s0)
                    nc.gpsimd.dma_gather(
                        out_ap=gB3[:, s0 : s0 + sn],
                        in_ap=htab[HIBASE:],
                        idxs_ap=it_t[
                            :, i0 + 8 * (na + s0) : i0 + 8 * (na + s0 + sn)
                        ],
                        num_idxs=128 * sn,
                        num_idxs_reg=128 * sn,
                        elem_size=HID,
                        single_packet=SINGLE_PACKET,
                        queue_num=qrr[0] % 4,
                    )
                    qrr[0] += 1
                # t = tanh(z/2); ex = exp(t/2)  (softmax scale e^0.5 cancels)
                twA = spool.tile([128, na], F32, tag="twA")
                nc.scalar.activation(
                    twA[:], esw_t[:, a0:a1], AF.Tanh, scale=0.5
                )
                ex2A = spool.tile([128, na, 2], F16, tag="ex2A")
                nc.scalar.activation(
                    ex2A[:],
                    twA[:, :, None].to_broadcast([128, na, 2]),
                    AF.Exp,
                    scale=0.5,
                )
                gsA = mpool.tile([128, na * HID], F16, tag="gsA")
                nc.vector.tensor_tensor(
                    out=gsA[:].rearrange("p (c a two) -> p c a two", a=64, two=2),
                    in0=gA3.rearrange("p c (a two) -> p c a two", two=2),
                    in1=ex2A[:, :, None, :].to_broadcast([128, na, 64, 2]),
                    op=ALU.mult,
                )
                if nb_:
                    twB = spool.tile([128, nb_], F32, tag="twB")
                    nc.scalar.activation(
                        twB[:], esw_t[:, TOTA + b0 : TOTA + b1], AF.Tanh,
                        scale=0.5,
                    )
                    ex2B = spool.tile([128, nb_, 2], F16, tag="ex2B")
                    nc.scalar.activation(
                        ex2B[:],
                        twB[:, :, None].to_broadcast([128, nb_, 2]),
                        AF.Exp,
                        scale=0.5,
                    )
                    gsB = mpool.tile([128, nb_ * HID], F16, tag="gsB")
                    nc.vector.tensor_tensor(
                        out=gsB[:].rearrange(
                            "p (c a two) -> p c a two", a=64, two=2
                        ),
                        in0=gB3[:, :nb_].rearrange(
                            "p c (a two) -> p c a two", two=2
                        ),
                        in1=ex2B[:, :, None, :].to_broadcast([128, nb_, 64, 2]),
                        op=ALU.mult,
                    )
                for w in range(w0, w1_):
                    ncA = int(nchA[w])
                    ncB = int(nchB[w])
                    loA = int(offsA[w]) - a0
                    loB = int(offsB[w]) - b0
                    den = spool.tile([128, 1], F32, tag="den")
                    nc.vector.reduce_sum(
                        den[:], ex2A[:, loA : loA + ncA, 0],
                        axis=mybir.AxisListType.X,
                    )
                    if ncB:
                        denB = spool.tile([128, 1], F32, tag="denB")
                        nc.vector.reduce_sum(
                            denB[:], ex2B[:, loB : loB + ncB, 0],
                            axis=mybir.AxisListType.X,
                        )
                        nc.vector.tensor_tensor(
                            out=den[:], in0=den[:], in1=denB[:], op=ALU.add
                        )
                    nc.vector.tensor_scalar(
                        out=den[:], in0=den[:], scalar1=pcw_t[:, w : w + 1],
                        scalar2=0.5, op0=ALU.subtract, op1=ALU.max,
                    )
                    recip = spool.tile([128, 1], F32, tag="recip")
                    nc.vector.reciprocal(recip[:], den[:])
                    # 4-chunk-wide identity matmuls into 4 PSUM lanes,
                    # then one strided DVE reduce lane-sum.
                    acc4 = psum.tile([128, 4 * HID], F32, tag="acc4")
                    lanes = min(4, max(ncA, ncB))
                    sides = [(ncA, loA, gsA), (ncB, loB, gsB)]
                    if ncB > ncA:
                        sides.reverse()
                    pieces = [
                        (loS, gsS, c0_, min(4, ncS - c0_))
                        for (ncS, loS, gsS) in sides
                        for c0_ in range(0, ncS, 4)
                    ]
                    for pi, (loS, gsS, c0_, wd) in enumerate(pieces):
                        nc.tensor.matmul(
                            out=acc4[:, : wd * HID],
                            lhsT=ident[:],
                            rhs=gsS[
                                :, (loS + c0_) * HID : (loS + c0_ + wd) * HID
                            ],
                            start=(pi == 0),
                            stop=(pi == len(pieces) - 1),
                            skip_group_check=True,
                        )
                    accr = spool.tile([128, HID], F32, tag="accr")
                    nc.vector.reduce_sum(
                        accr[:],
                        acc4[:].rearrange("p (l f) -> p f l", l=4)[:, :, :lanes],
                        axis=mybir.AxisListType.X,
                    )
                    # ELU+1: max(x,0) + exp(min(x,0)), x = accr*recip
                    mm = spool.tile([128, HID], F32, tag="mm")
                    nc.vector.tensor_scalar(
                        out=mm[:], in0=accr[:], scalar1=recip[:],
                        scalar2=0.0, op0=ALU.mult, op1=ALU.min,
                    )
                    rr = spool.tile([128, HID], F32, tag="rr")
                    nc.vector.tensor_scalar(
                        out=rr[:], in0=accr[:], scalar1=recip[:],
                        scalar2=0.0, op0=ALU.mult, op1=ALU.max,
                    )
                    ee = spool.tile([128, HID], F32, tag="ee")
                    nc.scalar.activation(ee[:], mm[:], AF.Exp)
                    h1 = hpool.tile([128, HID], F16, tag="h1")
                    nc.vector.tensor_tensor(
                        out=h1[:], in0=rr[:], in1=ee[:], op=ALU.add
                    )
                    # yT_w = W2^T @ h1^T - colsum(W2) (the ELU -1 term)
                    h1tp = psumt.tile([128, HID], F16, tag="h1tp")
                    nc.tensor.transpose(
                        out=h1tp[:], in_=h1[:], identity=ident[:]
                    )
                    h1t = hpool.tile([128, HID], F16, tag="h1t")
                    nc.scalar.activation(h1t[:], h1tp[:], AF.Copy)
                    yp = psumy.tile([OUT, 128], F32, tag="yp")
                    nc.tensor.matmul(
                        out=yp[:], lhsT=w2_t[:], rhs=h1t[:],
                        start=True, stop=True,
                    )
                    yt = hpool.tile([OUT, 128], F16, tag="yt")
                    nc.scalar.activation(
                        yt[:], yp[:], AF.Identity, bias=c2n_t[:]
                    )
                    nc.sync.dma_start(
                        out=yT[:, w * 128 : (w + 1) * 128], in_=yt[:]
                    )
    nc.finalize()
    return nc


# ---------------------------------------------------------------- host glue
def _wrap16(idx_cols):
    """[128, ncols] int16 slot indices -> dma_gather wrapped layout.

    Flattened order i = c*128 + p; idx i lives at (partition i%16,
    col i//16), replicated across the 8 groups of 16 partitions.
    Returns [128, 8*ncols]."""
    ncols = idx_cols.shape[1]
    flat = idx_cols.T.reshape(-1)                 # i = c*128 + p
    wrapped = flat.reshape(8 * ncols, 16).T       # [16, 8*ncols]
    return np.tile(wrapped, (8, 1)).astype(np.int16)


def kernel(x, edge_index, W1, att_src, att_dst, W2):
    x = np.asarray(x, dtype=np.float32)
    edge_index = np.asarray(edge_index)
    W1 = np.asarray(W1, dtype=np.float32)
    att_src = np.asarray(att_src, dtype=np.float32)
    att_dst = np.asarray(att_dst, dtype=np.float32)
    W2 = np.asarray(W2, dtype=np.float32)

    src = edge_index[0].astype(np.int64)
    dst = edge_index[1].astype(np.int64)

    # ---- phase 1: sharded h/es/ed compute
    wcat = np.concatenate(
        [W1, (W1 @ att_src)[:, None], (W1 @ att_dst)[:, None]], axis=1
    ).astype(np.float16)
    xT = np.ascontiguousarray(x.T).astype(np.float16)  # [IN, N]

    nc1 = build_phase1()
    in_maps1 = []
    for c in range(NCORES):
        sh = xT[:, c * NPC : (c + 1) * NPC]
        if sh.shape[1] < NPAD:
            sh = np.concatenate(
                [sh, np.zeros((IN, NPAD - sh.shape[1]), np.float16)], axis=1
            )
        in_maps1.append({"xT": np.ascontiguousarray(sh), "wcat": wcat})
    trace = os.environ.get("BASS_GAT_TRACE") == "1"
    tkw = dict(trace=True, trace_cores=[0]) if trace else {}
    if trace:
        _patch_perfetto()
    t0 = time.time()
    res1 = run_bass_kernel_spmd(nc1, in_maps1, core_ids=list(range(NCORES)), **tkw)
    _timings["phase1_wall"] = time.time() - t0
    _timings["phase1_ns"] = res1.exec_time_ns

    htab = np.zeros((TABROWS, HID), np.float16)
    es_full = np.zeros(N, np.float32)
    ed_full = np.zeros(N, np.float32)
    for c in range(NCORES):
        hv = res1.results[c]["haug"][:NPC]
        htab[1 + c * NPC : 1 + (c + 1) * NPC] = hv[:, :HID]
        es_full[c * NPC : (c + 1) * NPC] = hv[:, HID].astype(np.float32)
        ed_full[c * NPC : (c + 1) * NPC] = hv[:, HID + 1].astype(np.float32)

    # ---- host edge routing
    deg = np.bincount(dst, minlength=N)
    is_low = (src + 1) < LOWROWS
    degA_full = np.bincount(dst[is_low], minlength=N)
    degB_full = deg - degA_full

    # per-node low/high src lists (dst-sorted edge order)
    lkey = dst * 2 + (~is_low).astype(np.int64)   # low edges first per dst
    eorder = np.argsort(lkey, kind="stable")
    src_s = src[eorder]
    estarts = np.zeros(N + 1, np.int64)
    estarts[1:] = np.cumsum(deg)

    orders = []
    nchA_pc = np.zeros((NCORES, NW), np.int64)
    nchB_pc = np.zeros((NCORES, NW), np.int64)
    for c in range(NCORES):
        sl = slice(c * NPC, (c + 1) * NPC)
        dA, dB = degA_full[sl], degB_full[sl]
        order = np.lexsort((-dB, -dA))
        orders.append(order)
        dAs, dBs = dA[order], dB[order]
        for w in range(NW):
            j0 = w * 128
            if j0 < NPC:
                j1 = min(j0 + 128, NPC)
                nchA_pc[c, w] = dAs[j0:j1].max()
                nchB_pc[c, w] = dBs[j0:j1].max()
    nchA = np.maximum(nchA_pc.max(axis=0), 1)
    nchB = nchB_pc.max(axis=0)
    offsA = np.zeros(NW + 1, np.int64)
    offsA[1:] = np.cumsum(nchA)
    offsB = np.zeros(NW + 1, np.int64)
    offsB[1:] = np.cumsum(nchB)
    TOTA, TOTB = int(offsA[-1]), int(offsB[-1])

    groups = []
    w0 = 0
    while w0 < NW:
        w1_ = w0 + 1
        while w1_ < NW and (
            (offsA[w1_ + 1] - offsA[w0]) + (offsB[w1_ + 1] - offsB[w0])
            <= GCOLS
        ):
            w1_ += 1
        groups.append((w0, w1_))
        w0 = w1_

    in_maps2 = []
    for c in range(NCORES):
        order = orders[c]
        idxA = np.zeros((128, TOTA), np.int64)            # dummy low row 0
        idxB = np.full((128, TOTB), DUMMY_HI_LOCAL, np.int64)
        esw_arr = np.full((128, TOTA + TOTB), -60000.0, np.float32)
        padcnt = np.zeros((128, NW), np.float32)
        for wloc in range(NW):
            j0 = wloc * 128
            nodes = order[j0 : j0 + 128]
            ncA, ncB = int(nchA[wloc]), int(nchB[wloc])
            for p, j in enumerate(nodes):
                g = c * NPC + j
                dA = int(degA_full[g])
                dB = int(degB_full[g])
                s0 = estarts[g]
                ssA = src_s[s0 : s0 + dA]               # low edges first
                ssB = src_s[s0 + dA : s0 + dA + dB]
                colA = int(offsA[wloc])
                colB = int(offsB[wloc])
                idxA[p, colA : colA + dA] = ssA + 1
                idxB[p, colB : colB + dB] = ssB + 1 - HIBASE
                esw_arr[p, colA : colA + dA] = es_full[ssA] + ed_full[g]
                esw_arr[p, colA + dA : colA + ncA] += ed_full[g]
                esw_arr[p, TOTA + colB : TOTA + colB + dB] = (
                    es_full[ssB] + ed_full[g]
                )
                esw_arr[p, TOTA + colB + dB : TOTA + colB + ncB] += ed_full[g]
                padcnt[p, wloc] = (ncA - dA) + (ncB - dB)
            for p in range(len(nodes), 128):
                padcnt[p, wloc] = ncA + ncB
        iparts = []
        for (gw0, gw1) in groups:
            iparts.append(
                _wrap16(idxA[:, offsA[gw0] : offsA[gw1]].astype(np.int16))
            )
            if offsB[gw1] > offsB[gw0]:
                iparts.append(
                    _wrap16(idxB[:, offsB[gw0] : offsB[gw1]].astype(np.int16))
                )
        idxs_full = np.ascontiguousarray(np.concatenate(iparts, axis=1))
        in_maps2.append(
            {
                "htab": htab,
                "idxs": idxs_full,
                "esw": esw_arr.astype(np.float16),
                "pcw": (padcnt * np.float32(np.exp(-0.5))).astype(np.float32),
                "w2": W2.astype(np.float16),
                "c2n": -W2.sum(axis=0, dtype=np.float32)[:, None],
            }
        )

    nc2 = build_phase2(nchA, nchB, groups)
    t0 = time.time()
    res2 = run_bass_kernel_spmd(nc2, in_maps2, core_ids=list(range(NCORES)), **tkw)
    _timings["phase2_wall"] = time.time() - t0
    _timings["phase2_ns"] = res2.exec_time_ns

    out = np.zeros((N, OUT), np.float32)
    for c in range(NCORES):
        yv = res2.results[c]["yT"].astype(np.float32).T  # [NPAD, OUT]
        order = orders[c]
        out[c * NPC + order] = yv[:NPC]
    return out


# revision 13
# speedup vs baseline: 1.2797x; 1.2797x over previous
"""GAT encoder (gnn_message_passing) on 8 trn2 NeuronCores via Bass.

Strategy (graph-parallel, dst-sharded), v2:
  Phase 1 (node-sharded): one fp16 matmul chain per 128-node tile against
    [W1 | W1@att_src | W1@att_dst] -> rows [h(128) | es | ed] fp16.
  Host: build gather table htab (h rows, 256B fp16), route edges to
    dst-owner cores into windows of 128 dst nodes sorted by (low-degree,
    high-degree); per-slot es+ed bias; per-window pad counts.
  Phase 2 (per core): dma_gather bulk-fetches edge-slot h rows.  int16
    indices limit a gather table to 32768 rows, so each window has
    nchA low-src columns + nchB high-src columns, fetched by two calls
    per group from the two table halves (pad slots read a zero dummy row;
    every index is valid - HW emits descriptors for the first
    num_idxs_reg entries and skips nothing).  Attention without act-table
    swaps: exp(sigmoid(z)) = e^0.5 * exp(tanh(z/2)/2); the constant
    cancels in the softmax (pad slots use es=-60000 -> e^-0.5, removed
    from the denominator via pcw).  ex applied via one DVE fp16 multiply
    per pass per group; per-window PSUM accumulation via fp16 identity
    matmuls; ELU via max(x,0)+exp(min(x,0))-1 with the -1 folded into
    the output bias; y produced transposed and fixed up on host.
"""
import os
import sys
import time

sys.path.insert(0, "/opt/trn_rl_repo")

import numpy as np

N, E = 50000, 800000
IN, HID, OUT = 256, 128, 128
NCORES = 8
NPC = N // NCORES            # nodes per core (6250)
NW = (NPC + 127) // 128      # 49 windows; last partial (6250 = 48*128+106)
NPAD = NW * 128              # 6272
GCOLS = int(os.environ.get("BASS_GAT_GCOLS", "80"))  # max gather cols/group
CALLCOLS = int(os.environ.get("BASS_GAT_CALLCOLS", "64"))
SINGLE_PACKET = os.environ.get("BASS_GAT_SP", "0") == "1"

TABROWS = N + 3              # row 0 = zero (low dummy), 1..N = node src+1,
LOWROWS = 32768              # row N+2 = zero (high dummy)
HIBASE = 32767
DUMMY_HI_LOCAL = N + 2 - HIBASE

_timings = {}


def _patch_env():
    """Tile/perfetto compatibility patches for this container."""
    import concourse.tile as tile
    from concourse.tile import ScopedClock

    def _drain_and_barrier_split(self, tick_clock, wait_clock):
        nc = self.nc
        probe = nc.sync.nop()
        wait_clock.add_sem_waits(
            probe.ins, ScopedClock({None: tick_clock.global_clock})
        )
        waits = list(probe.ins.sync_info.on_wait or [])
        probe.ins.sync_info.on_wait = []
        from concourse import mybir

        for w in waits:
            inst = nc.sync.nop()
            if inst.ins.sync_info is None:
                inst.ins.sync_info = mybir.SyncInfo(on_wait=[w], on_update=[])
            else:
                inst.ins.sync_info.on_wait = [w]
        nc.sync.drain()
        nc.all_engine_barrier()
        assert self.sems is not None
        popped = nc._tile_sem_poison_stack.pop()
        assert popped is self._sem_poison
        nc.clear_and_free_semaphores(list(self.sems.allocated().values()))
        nc.all_engine_barrier()

    tile.TileContext._drain_and_barrier = _drain_and_barrier_split


_patch_env()


def _patch_perfetto():
    try:
        from gauge import trn_perfetto

        cls = trn_perfetto.TrnPerfettoConv
        if not getattr(cls, "_no_hlo_patched", False):
            _orig_init = cls.__init__

            def _init_no_hlo(self, *a, **k):
                k["annotate_hlo"] = False
                if len(a) >= 2:
                    a = (a[0], False) + a[2:]
                _orig_init(self, *a, **k)

            cls.__init__ = _init_no_hlo
            cls._no_hlo_patched = True
    except Exception:
        pass


import concourse.bass as bass
import concourse.bacc as bacc
import concourse.tile as tile
from concourse import mybir
from concourse.bass_utils import run_bass_kernel_spmd
from concourse.masks import make_identity

F32 = mybir.dt.float32
F16 = mybir.dt.float16
I16 = mybir.dt.int16
AF = mybir.ActivationFunctionType
ALU = mybir.AluOpType


# ---------------------------------------------------------------- phase 1
def build_phase1():
    """h/es/ed for this core's nodes: one fp16 matmul chain per tile."""
    nc = bacc.Bacc("TRN2", target_bir_lowering=True)
    xT = nc.dram_tensor("xT", [IN, NPAD], F16, kind="ExternalInput")
    wcat = nc.dram_tensor("wcat", [IN, HID + 2], F16, kind="ExternalInput")
    haug = nc.dram_tensor("haug", [NPAD, HID + 2], F16, kind="ExternalOutput")

    RF = HID + 2  # 130
    TB = 4        # tiles per DMA batch

    with tile.TileContext(nc) as tc:
        with (
            tc.tile_pool(name="sbuf", bufs=3) as pool,
            tc.tile_pool(name="cpool", bufs=1) as cpool,
            tc.tile_pool(name="psum", bufs=4, space="PSUM") as psum,
        ):
            w_t = cpool.tile([128, IN // 128, RF], F16)
            nc.sync.dma_start(
                out=w_t[:], in_=wcat[:].rearrange("(a k) f -> k a f", k=128)
            )
            nb = (NW + TB - 1) // TB
            for b in range(nb):
                t0 = b * TB
                tn = min(TB, NW - t0)
                xt = pool.tile([128, IN // 128, TB * 128], F16, tag="xt")
                nc.sync.dma_start(
                    out=xt[:, :, : tn * 128],
                    in_=xT[:, t0 * 128 : (t0 + tn) * 128].rearrange(
                        "(a k) n -> k a n", k=128
                    ),
                )
                ha = pool.tile([128, TB, RF], F16, tag="ha")
                for t in range(tn):
                    hp = psum.tile([128, RF], F32, tag="hp")
                    for a in range(IN // 128):
                        nc.tensor.matmul(
                            out=hp[:],
                            lhsT=xt[:, a, t * 128 : (t + 1) * 128],
                            rhs=w_t[:, a],
                            start=(a == 0),
                            stop=(a == IN // 128 - 1),
                        )
                    nc.scalar.activation(ha[:, t], hp[:], AF.Copy)
                nc.sync.dma_start(
                    out=haug[t0 * 128 : (t0 + tn) * 128, :].rearrange(
                        "(t k) f -> k t f", k=128
                    ),
                    in_=ha[:, :tn],
                )
    nc.finalize()
    return nc


# ---------------------------------------------------------------- phase 2
def build_phase2(nchA, nchB, groups):
    """Per-window low/high chunk counts and group spans."""
    NWl = len(nchA)
    offsA = np.zeros(NWl + 1, dtype=int)
    offsA[1:] = np.cumsum(nchA)
    offsB = np.zeros(NWl + 1, dtype=int)
    offsB[1:] = np.cumsum(nchB)
    TOTA, TOTB = int(offsA[-1]), int(offsB[-1])
    # idx segment start per group (8 wrapped cols per gather col, A then B)
    ico = np.zeros(len(groups) + 1, dtype=int)
    for g, (w0, w1_) in enumerate(groups):
        na = int(offsA[w1_] - offsA[w0])
        nb_ = int(offsB[w1_] - offsB[w0])
        ico[g + 1] = ico[g] + 8 * (na + nb_)
    ITOT = int(ico[-1])

    nc = bacc.Bacc(
        "TRN2",
        target_bir_lowering=True,
        num_swdge_queues=4,
        dynamic_dma_scratch_size=int(
            os.environ.get("BASS_GAT_SCRATCH", "16384")
        ),
    )
    htab = nc.dram_tensor("htab", [TABROWS, HID], F16, kind="ExternalInput")
    idxs = nc.dram_tensor("idxs", [128, ITOT], I16, kind="ExternalInput")
    esw = nc.dram_tensor("esw", [128, TOTA + TOTB], F16, kind="ExternalInput")
    pcw = nc.dram_tensor("pcw", [128, NW], F32, kind="ExternalInput")
    w2 = nc.dram_tensor("w2", [HID, OUT], F16, kind="ExternalInput")
    c2n = nc.dram_tensor("c2n", [OUT, 1], F32, kind="ExternalInput")
    yT = nc.dram_tensor("yT", [OUT, NW * 128], F16, kind="ExternalOutput")

    with tile.TileContext(nc) as tc:
        with (
            tc.tile_pool(name="gpool", bufs=4) as gpool,
            tc.tile_pool(name="mpool", bufs=3) as mpool,
            tc.tile_pool(name="spool", bufs=6) as spool,
            tc.tile_pool(name="hpool", bufs=3) as hpool,
            tc.tile_pool(name="cpool", bufs=1) as cpool,
            tc.tile_pool(name="psum", bufs=4, space="PSUM") as psum,
            tc.tile_pool(name="psumt", bufs=2, space="PSUM") as psumt,
            tc.tile_pool(name="psumy", bufs=2, space="PSUM") as psumy,
        ):
            ident = cpool.tile([128, 128], F16)
            make_identity(nc, ident[:])
            w2_t = cpool.tile([HID, OUT], F16)
            nc.sync.dma_start(out=w2_t[:], in_=w2[:])
            c2n_t = cpool.tile([OUT, 1], F32)
            nc.sync.dma_start(out=c2n_t[:], in_=c2n[:])
            pcw_t = cpool.tile([128, NW], F32)
            nc.sync.dma_start(out=pcw_t[:], in_=pcw[:])
            it_t = cpool.tile([128, ITOT], I16)
            nc.sync.dma_start(out=it_t[:], in_=idxs[:])
            qrr = [0]
            esw_t = cpool.tile([128, TOTA + TOTB], F16)
            nc.sync.dma_start(out=esw_t[:], in_=esw[:])

            # largest groups first: the pipeline drains on a small one
            for g, (w0, w1_) in reversed(list(enumerate(groups))):
                a0, a1 = int(offsA[w0]), int(offsA[w1_])
                b0, b1 = int(offsB[w0]), int(offsB[w1_])
                na, nb_ = a1 - a0, b1 - b0
                i0 = int(ico[g])

                gtA = gpool.tile([128, na * HID], F16, tag="gtA")
                gA3 = gtA[:].rearrange("p (c f) -> p c f", f=HID)
                for s0 in range(0, na, CALLCOLS):
                    sn = min(CALLCOLS, na - s0)
                    nc.gpsimd.dma_gather(
                        out_ap=gA3[:, s0 : s0 + sn],
                        in_ap=htab[:LOWROWS],
                        idxs_ap=it_t[:, i0 + 8 * s0 : i0 + 8 * (s0 + sn)],
                        num_idxs=128 * sn,
                        num_idxs_reg=128 * sn,
                        elem_size=HID,
                        single_packet=SINGLE_PACKET,
                        queue_num=qrr[0] % 4,
                    )
                    qrr[0] += 1
                gtB = gpool.tile([128, max(nb_, 1) * HID], F16, tag="gtB")
                gB3 = gtB[:].rearrange("p (c f) -> p c f", f=HID)
                for s0 in range(0, nb_, CALLCOLS):
                    sn = min(CALLCOLS, nb_ - s0)
                    nc.gpsimd.dma_gather(
                        out_ap=gB3[:, s0 : s0 + sn],
                        in_ap=htab[HIBASE:],
                        idxs_ap=it_t[
                            :, i0 + 8 * (na + s0) : i0 + 8 * (na + s0 + sn)
                        ],
                        num_idxs=128 * sn,
                        num_idxs_reg=128 * sn,
                        elem_size=HID,
                        single_packet=SINGLE_PACKET,
                        queue_num=qrr[0] % 4,
                    )
                    qrr[0] += 1
                # t = tanh(z/2); ex = exp(t/2)  (softmax scale e^0.5 cancels)
                twA = spool.tile([128, na], F32, tag="twA")
                nc.scalar.activation(
                    twA[:], esw_t[:, a0:a1], AF.Tanh, scale=0.5
                )
                ex2A = spool.tile([128, na, 2], F16, tag="ex2A")
                nc.scalar.activation(
                    ex2A[:],
                    twA[:, :, None].to_broadcast([128, na, 2]),
                    AF.Exp,
                    scale=0.5,
                )
                gsA = mpool.tile([128, na * HID], F16, tag="gsA")
                nc.vector.tensor_tensor(
                    out=gsA[:].rearrange("p (c a two) -> p c a two", a=64, two=2),
                    in0=gA3.rearrange("p c (a two) -> p c a two", two=2),
                    in1=ex2A[:, :, None, :].to_broadcast([128, na, 64, 2]),
                    op=ALU.mult,
                )
                if nb_:
                    twB = spool.tile([128, nb_], F32, tag="twB")
                    nc.scalar.activation(
                        twB[:], esw_t[:, TOTA + b0 : TOTA + b1], AF.Tanh,
                        scale=0.5,
                    )
                    ex2B = spool.tile([128, nb_, 2], F16, tag="ex2B")
                    nc.scalar.activation(
                        ex2B[:],
                        twB[:, :, None].to_broadcast([128, nb_, 2]),
                        AF.Exp,
                        scale=0.5,
                    )
                    gsB = mpool.tile([128, nb_ * HID], F16, tag="gsB")
                    nc.vector.tensor_tensor(
                        out=gsB[:].rearrange(
                            "p (c a two) -> p c a two", a=64, two=2
                        ),
                        in0=gB3[:, :nb_].rearrange(
                            "p c (a two) -> p c a two", two=2
                        ),
                        in1=ex2B[:, :, None, :].to_broadcast([128, nb_, 64, 2]),
                        op=ALU.mult,
                    )
                for w in range(w0, w1_):
                    ncA = int(nchA[w])
                    ncB = int(nchB[w])
                    loA = int(offsA[w]) - a0
                    loB = int(offsB[w]) - b0
                    den = spool.tile([128, 1], F32, tag="den")
                    nc.vector.reduce_sum(
                        den[:], ex2A[:, loA : loA + ncA, 0],
                        axis=mybir.AxisListType.X,
                    )
                    if ncB:
                        denB = spool.tile([128, 1], F32, tag="denB")
                        nc.vector.reduce_sum(
                            denB[:], ex2B[:, loB : loB + ncB, 0],
                            axis=mybir.AxisListType.X,
                        )
                        nc.vector.tensor_tensor(
                            out=den[:], in0=den[:], in1=denB[:], op=ALU.add
                        )
                    nc.vector.tensor_scalar(
                        out=den[:], in0=den[:], scalar1=pcw_t[:, w : w + 1],
                        scalar2=0.5, op0=ALU.subtract, op1=ALU.max,
                    )
                    recip = spool.tile([128, 1], F32, tag="recip")
                    nc.vector.reciprocal(recip[:], den[:])
                    acc = psum.tile([128, HID], F32, tag="acc")
                    ncht = ncA + ncB
                    for c in range(ncA):
                        nc.tensor.matmul(
                            out=acc[:],
                            lhsT=ident[:],
                            rhs=gsA[:, (loA + c) * HID : (loA + c + 1) * HID],
                            start=(c == 0),
                            stop=(c == ncht - 1),
                        )
                    for c in range(ncB):
                        nc.tensor.matmul(
                            out=acc[:],
                            lhsT=ident[:],
                            rhs=gsB[:, (loB + c) * HID : (loB + c + 1) * HID],
                            start=False,
                            stop=(ncA + c == ncht - 1),
                        )
                    # ELU+1: max(x,0) + exp(min(x,0)), x = acc*recip
                    mm = spool.tile([128, HID], F32, tag="mm")
                    nc.vector.tensor_scalar(
                        out=mm[:], in0=acc[:], scalar1=recip[:],
                        scalar2=0.0, op0=ALU.mult, op1=ALU.min,
                    )
                    rr = spool.tile([128, HID], F32, tag="rr")
                    nc.vector.tensor_scalar(
                        out=rr[:], in0=acc[:], scalar1=recip[:],
                        scalar2=0.0, op0=ALU.mult, op1=ALU.max,
                    )
                    ee = spool.tile([128, HID], F32, tag="ee")
                    nc.scalar.activation(ee[:], mm[:], AF.Exp)
                    h1 = hpool.tile([128, HID], F16, tag="h1")
                    nc.vector.tensor_tensor(
                        out=h1[:], in0=rr[:], in1=ee[:], op=ALU.add
                    )
                    # yT_w = W2^T @ h1^T - colsum(W2) (the ELU -1 term)
                    h1tp = psumt.tile([128, HID], F16, tag="h1tp")
                    nc.tensor.transpose(
                        out=h1tp[:], in_=h1[:], identity=ident[:]
                    )
                    h1t = hpool.tile([128, HID], F16, tag="h1t")
                    nc.scalar.activation(h1t[:], h1tp[:], AF.Copy)
                    yp = psumy.tile([OUT, 128], F32, tag="yp")
                    nc.tensor.matmul(
                        out=yp[:], lhsT=w2_t[:], rhs=h1t[:],
                        start=True, stop=True,
                    )
                    yt = hpool.tile([OUT, 128], F16, tag="yt")
                    nc.scalar.activation(
                        yt[:], yp[:], AF.Identity, bias=c2n_t[:]
                    )
                    nc.sync.dma_start(
                        out=yT[:, w * 128 : (w + 1) * 128], in_=yt[:]
                    )
    nc.finalize()
    return nc


# ---------------------------------------------------------------- host glue
def _wrap16(idx_cols):
    """[128, ncols] int16 slot indices -> dma_gather wrapped layout.

    Flattened order i = c*128 + p; idx i lives at (partition i%16,
    col i//16), replicated across the 8 groups of 16 partitions.
    Returns [128, 8*ncols]."""
    ncols = idx_cols.shape[1]
    flat = idx_cols.T.reshape(-1)                 # i = c*128 + p
    wrapped = flat.reshape(8 * ncols, 16).T       # [16, 8*ncols]
    return np.tile(wrapped, (8, 1)).astype(np.int16)


def kernel(x, edge_index, W1, att_src, att_dst, W2):
    x = np.asarray(x, dtype=np.float32)
    edge_index = np.asarray(edge_index)
    W1 = np.asarray(W1, dtype=np.float32)
    att_src = np.asarray(att_src, dtype=np.float32)
    att_dst = np.asarray(att_dst, dtype=np.float32)
    W2 = np.asarray(W2, dtype=np.float32)

    src = edge_index[0].astype(np.int64)
    dst = edge_index[1].astype(np.int64)

    # ---- phase 1: sharded h/es/ed compute
    wcat = np.concatenate(
        [W1, (W1 @ att_src)[:, None], (W1 @ att_dst)[:, None]], axis=1
    ).astype(np.float16)
    xT = np.ascontiguousarray(x.T).astype(np.float16)  # [IN, N]

    nc1 = build_phase1()
    in_maps1 = []
    for c in range(NCORES):
        sh = xT[:, c * NPC : (c + 1) * NPC]
        if sh.shape[1] < NPAD:
            sh = np.concatenate(
                [sh, np.zeros((IN, NPAD - sh.shape[1]), np.float16)], axis=1
            )
        in_maps1.append({"xT": np.ascontiguousarray(sh), "wcat": wcat})
    trace = os.environ.get("BASS_GAT_TRACE") == "1"
    tkw = dict(trace=True, trace_cores=[0]) if trace else {}
    if trace:
        _patch_perfetto()
    t0 = time.time()
    res1 = run_bass_kernel_spmd(nc1, in_maps1, core_ids=list(range(NCORES)), **tkw)
    _timings["phase1_wall"] = time.time() - t0
    _timings["phase1_ns"] = res1.exec_time_ns

    htab = np.zeros((TABROWS, HID), np.float16)
    es_full = np.zeros(N, np.float32)
    ed_full = np.zeros(N, np.float32)
    for c in range(NCORES):
        hv = res1.results[c]["haug"][:NPC]
        htab[1 + c * NPC : 1 + (c + 1) * NPC] = hv[:, :HID]
        es_full[c * NPC : (c + 1) * NPC] = hv[:, HID].astype(np.float32)
        ed_full[c * NPC : (c + 1) * NPC] = hv[:, HID + 1].astype(np.float32)

    # ---- host edge routing
    deg = np.bincount(dst, minlength=N)
    is_low = (src + 1) < LOWROWS
    degA_full = np.bincount(dst[is_low], minlength=N)
    degB_full = deg - degA_full

    # per-node low/high src lists (dst-sorted edge order)
    lkey = dst * 2 + (~is_low).astype(np.int64)   # low edges first per dst
    eorder = np.argsort(lkey, kind="stable")
    src_s = src[eorder]
    estarts = np.zeros(N + 1, np.int64)
    estarts[1:] = np.cumsum(deg)

    orders = []
    nchA_pc = np.zeros((NCORES, NW), np.int64)
    nchB_pc = np.zeros((NCORES, NW), np.int64)
    for c in range(NCORES):
        sl = slice(c * NPC, (c + 1) * NPC)
        dA, dB = degA_full[sl], degB_full[sl]
        order = np.lexsort((-dB, -dA))
        orders.append(order)
        dAs, dBs = dA[order], dB[order]
        for w in range(NW):
            j0 = w * 128
            if j0 < NPC:
                j1 = min(j0 + 128, NPC)
                nchA_pc[c, w] = dAs[j0:j1].max()
                nchB_pc[c, w] = dBs[j0:j1].max()
    nchA = np.maximum(nchA_pc.max(axis=0), 1)
    nchB = nchB_pc.max(axis=0)
    offsA = np.zeros(NW + 1, np.int64)
    offsA[1:] = np.cumsum(nchA)
    offsB = np.zeros(NW + 1, np.int64)
    offsB[1:] = np.cumsum(nchB)
    TOTA, TOTB = int(offsA[-1]), int(offsB[-1])

    groups = []
    w0 = 0
    while w0 < NW:
        w1_ = w0 + 1
        while w1_ < NW and (
            (offsA[w1_ + 1] - offsA[w0]) + (offsB[w1_ + 1] - offsB[w0])
            <= GCOLS
        ):
            w1_ += 1
        groups.append((w0, w1_))
        w0 = w1_

    in_maps2 = []
    for c in range(NCORES):
        order = orders[c]
        idxA = np.zeros((128, TOTA), np.int64)            # dummy low row 0
        idxB = np.full((128, TOTB), DUMMY_HI_LOCAL, np.int64)
        esw_arr = np.full((128, TOTA + TOTB), -60000.0, np.float32)
        padcnt = np.zeros((128, NW), np.float32)
        for wloc in range(NW):
            j0 = wloc * 128
            nodes = order[j0 : j0 + 128]
            ncA, ncB = int(nchA[wloc]), int(nchB[wloc])
            for p, j in enumerate(nodes):
                g = c * NPC + j
                dA = int(degA_full[g])
                dB = int(degB_full[g])
                s0 = estarts[g]
                ssA = src_s[s0 : s0 + dA]               # low edges first
                ssB = src_s[s0 + dA : s0 + dA + dB]
                colA = int(offsA[wloc])
                colB = int(offsB[wloc])
                idxA[p, colA : colA + dA] = ssA + 1
                idxB[p, colB : colB + dB] = ssB + 1 - HIBASE
                esw_arr[p, colA : colA + dA] = es_full[ssA] + ed_full[g]
                esw_arr[p, colA + dA : colA + ncA] += ed_full[g]
                esw_arr[p, TOTA + colB : TOTA + colB + dB] = (
                    es_full[ssB] + ed_full[g]
                )
                esw_arr[p, TOTA + colB + dB : TOTA + colB + ncB] += ed_full[g]
                padcnt[p, wloc] = (ncA - dA) + (ncB - dB)
            for p in range(len(nodes), 128):
                padcnt[p, wloc] = ncA + ncB
        iparts = []
        for (gw0, gw1) in groups:
            iparts.append(
                _wrap16(idxA[:, offsA[gw0] : offsA[gw1]].astype(np.int16))
            )
            if offsB[gw1] > offsB[gw0]:
                iparts.append(
                    _wrap16(idxB[:, offsB[gw0] : offsB[gw1]].astype(np.int16))
                )
        idxs_full = np.ascontiguousarray(np.concatenate(iparts, axis=1))
        in_maps2.append(
            {
                "htab": htab,
                "idxs": idxs_full,
                "esw": esw_arr.astype(np.float16),
                "pcw": (padcnt * np.float32(np.exp(-0.5))).astype(np.float32),
                "w2": W2.astype(np.float16),
                "c2n": -W2.sum(axis=0, dtype=np.float32)[:, None],
            }
        )

    nc2 = build_phase2(nchA, nchB, groups)
    t0 = time.time()
    res2 = run_bass_kernel_spmd(nc2, in_maps2, core_ids=list(range(NCORES)), **tkw)
    _timings["phase2_wall"] = time.time() - t0
    _timings["phase2_ns"] = res2.exec_time_ns

    out = np.zeros((N, OUT), np.float32)
    for c in range(NCORES):
        yv = res2.results[c]["yT"].astype(np.float32).T  # [NPAD, OUT]
        order = orders[c]
        out[c * NPC + order] = yv[:NPC]
    return out


# revision 14
# speedup vs baseline: 1.8350x; 1.4339x over previous
"""GAT encoder (gnn_message_passing) on 8 trn2 NeuronCores via Bass.

Strategy (graph-parallel, dst-sharded), v3:
  Phase 1 (node-sharded): one fp16 matmul chain per 128-node tile against
    [W1 | W1@att_src | W1@att_dst] -> rows [h(128) | es | ed] fp16.
  Host: build gather table htab (h rows, 256B fp16, row = src+1, zero
    dummy rows); int16 dma_gather indices limit a table to 32768 rows, so
    edges are split by src range into a LOW set (the bulk) and HIGH set.
  Phase 2 (per core), one launch, three sections:
    pass B: HIGH edges aggregated in their own dB-degree-sorted windows
      (near-zero padding); per window the weighted partial sum and partial
      denominator are written to a DRAM scratch table numB [6272, 256]B.
    main pass: LOW edges in dA-degree-sorted windows; each window's chunk
      list gets ONE extra chunk - the node's numB row fetched by a tiny
      per-group dma_gather - so high-src contributions merge in the PSUM
      accumulation; denominators add the gathered partial.
    Attention without act-table swaps: exp(sigmoid(z)) = e^.5*exp(tanh(z/2)/2);
    the constant cancels in the softmax (pad slots use es=-60000 -> e^-.5,
    removed via pcw).  ex applied by one DVE fp16 multiply per group;
    fp16 identity matmuls accumulate in PSUM; ELU via max(x,0)+exp(min(x,0))-1
    with the -1 folded into the output bias; y produced transposed.
  Gathers use 4 SWDGE queues round-robin (parallel Q7 descriptor gen).
"""
import os
import sys
import time

sys.path.insert(0, "/opt/trn_rl_repo")

import numpy as np

N, E = 50000, 800000
IN, HID, OUT = 256, 128, 128
NCORES = 8
NPC = N // NCORES            # nodes per core (6250)
NW = (NPC + 127) // 128      # 49 windows; last partial (6250 = 48*128+106)
NPAD = NW * 128              # 6272
GCOLS = int(os.environ.get("BASS_GAT_GCOLS", "64"))
CALLCOLS = int(os.environ.get("BASS_GAT_CALLCOLS", "16"))
SINGLE_PACKET = os.environ.get("BASS_GAT_SP", "0") == "1"
NBF = 2 * HID                # numB row: [acc f16 x128 | den | pad] = 512B

TABROWS = N + 3              # row 0 = zero (low dummy), 1..N = node src+1,
LOWROWS = 32768              # row N+2 = zero (high dummy)
HIBASE = 32767
DUMMY_HI_LOCAL = N + 2 - HIBASE

_timings = {}


def _patch_env():
    """Tile/perfetto compatibility patches for this container."""
    import concourse.tile as tile
    from concourse.tile import ScopedClock

    def _drain_and_barrier_split(self, tick_clock, wait_clock):
        nc = self.nc
        probe = nc.sync.nop()
        wait_clock.add_sem_waits(
            probe.ins, ScopedClock({None: tick_clock.global_clock})
        )
        waits = list(probe.ins.sync_info.on_wait or [])
        probe.ins.sync_info.on_wait = []
        from concourse import mybir

        for w in waits:
            inst = nc.sync.nop()
            if inst.ins.sync_info is None:
                inst.ins.sync_info = mybir.SyncInfo(on_wait=[w], on_update=[])
            else:
                inst.ins.sync_info.on_wait = [w]
        nc.sync.drain()
        nc.all_engine_barrier()
        assert self.sems is not None
        popped = nc._tile_sem_poison_stack.pop()
        assert popped is self._sem_poison
        nc.clear_and_free_semaphores(list(self.sems.allocated().values()))
        nc.all_engine_barrier()

    tile.TileContext._drain_and_barrier = _drain_and_barrier_split


_patch_env()


def _patch_perfetto():
    try:
        from gauge import trn_perfetto

        cls = trn_perfetto.TrnPerfettoConv
        if not getattr(cls, "_no_hlo_patched", False):
            _orig_init = cls.__init__

            def _init_no_hlo(self, *a, **k):
                k["annotate_hlo"] = False
                if len(a) >= 2:
                    a = (a[0], False) + a[2:]
                _orig_init(self, *a, **k)

            cls.__init__ = _init_no_hlo
            cls._no_hlo_patched = True
    except Exception:
        pass


import concourse.bass as bass
import concourse.bacc as bacc
import concourse.tile as tile
from concourse import mybir
from concourse.bass_utils import run_bass_kernel_spmd
from concourse.masks import make_identity

F32 = mybir.dt.float32
F16 = mybir.dt.float16
I16 = mybir.dt.int16
AF = mybir.ActivationFunctionType
ALU = mybir.AluOpType


# ---------------------------------------------------------------- phase 1
def build_phase1():
    """h/es/ed for this core's nodes: one fp16 matmul chain per tile."""
    nc = bacc.Bacc("TRN2", target_bir_lowering=True)
    xT = nc.dram_tensor("xT", [IN, NPAD], F16, kind="ExternalInput")
    wcat = nc.dram_tensor("wcat", [IN, HID + 2], F16, kind="ExternalInput")
    haug = nc.dram_tensor("haug", [NPAD, HID + 2], F16, kind="ExternalOutput")

    RF = HID + 2  # 130
    TB = 4        # tiles per DMA batch

    with tile.TileContext(nc) as tc:
        with (
            tc.tile_pool(name="sbuf", bufs=3) as pool,
            tc.tile_pool(name="cpool", bufs=1) as cpool,
            tc.tile_pool(name="psum", bufs=4, space="PSUM") as psum,
        ):
            w_t = cpool.tile([128, IN // 128, RF], F16)
            nc.sync.dma_start(
                out=w_t[:], in_=wcat[:].rearrange("(a k) f -> k a f", k=128)
            )
            nb = (NW + TB - 1) // TB
            for b in range(nb):
                t0 = b * TB
                tn = min(TB, NW - t0)
                xt = pool.tile([128, IN // 128, TB * 128], F16, tag="xt")
                nc.sync.dma_start(
                    out=xt[:, :, : tn * 128],
                    in_=xT[:, t0 * 128 : (t0 + tn) * 128].rearrange(
                        "(a k) n -> k a n", k=128
                    ),
                )
                ha = pool.tile([128, TB, RF], F16, tag="ha")
                for t in range(tn):
                    hp = psum.tile([128, RF], F32, tag="hp")
                    for a in range(IN // 128):
                        nc.tensor.matmul(
                            out=hp[:],
                            lhsT=xt[:, a, t * 128 : (t + 1) * 128],
                            rhs=w_t[:, a],
                            start=(a == 0),
                            stop=(a == IN // 128 - 1),
                        )
                    nc.scalar.activation(ha[:, t], hp[:], AF.Copy)
                nc.sync.dma_start(
                    out=haug[t0 * 128 : (t0 + tn) * 128, :].rearrange(
                        "(t k) f -> k t f", k=128
                    ),
                    in_=ha[:, :tn],
                )
    nc.finalize()
    return nc


# ---------------------------------------------------------------- phase 2
def build_phase2(nchA, nchB, groupsA, groupsB):
    """nchA/nchB: per-window chunk counts of the two window structures."""
    offsA = np.zeros(NW + 1, dtype=int)
    offsA[1:] = np.cumsum(nchA)
    offsB = np.zeros(NW + 1, dtype=int)
    offsB[1:] = np.cumsum(nchB)
    TOTA, TOTB = int(offsA[-1]), int(offsB[-1])
    # idx layout: B-group segs (8 cols per gather col), then A-group segs
    # (8*na + 8*nwin for the numB row gather)
    icoB = np.zeros(len(groupsB) + 1, dtype=int)
    for g, (w0, w1_) in enumerate(groupsB):
        icoB[g + 1] = icoB[g] + 8 * int(offsB[w1_] - offsB[w0])
    icoA = np.zeros(len(groupsA) + 1, dtype=int)
    icoA[0] = icoB[-1]
    for g, (w0, w1_) in enumerate(groupsA):
        icoA[g + 1] = icoA[g] + 8 * (int(offsA[w1_] - offsA[w0]) + (w1_ - w0))
    ITOT = int(icoA[-1])

    nc = bacc.Bacc(
        "TRN2",
        target_bir_lowering=True,
        num_swdge_queues=4,
        dynamic_dma_scratch_size=int(
            os.environ.get("BASS_GAT_SCRATCH", "16384")
        ),
    )
    htab = nc.dram_tensor("htab", [TABROWS, HID], F16, kind="ExternalInput")
    idxs = nc.dram_tensor("idxs", [128, ITOT], I16, kind="ExternalInput")
    esw = nc.dram_tensor("esw", [128, TOTA], F16, kind="ExternalInput")
    eswB = nc.dram_tensor("eswB", [128, TOTB], F16, kind="ExternalInput")
    pcw = nc.dram_tensor("pcw", [128, NW], F32, kind="ExternalInput")
    pcwB = nc.dram_tensor("pcwB", [128, NW], F32, kind="ExternalInput")
    w2 = nc.dram_tensor("w2", [HID, OUT], F16, kind="ExternalInput")
    c2n = nc.dram_tensor("c2n", [OUT, 1], F32, kind="ExternalInput")
    yT = nc.dram_tensor("yT", [OUT, NW * 128], F16, kind="ExternalOutput")

    qrr = [0]

    def gather(nc, out3, table_ap, it_t, i0, ncols):
        for s0 in range(0, ncols, CALLCOLS):
            sn = min(CALLCOLS, ncols - s0)
            nc.gpsimd.dma_gather(
                out_ap=out3[:, s0 : s0 + sn],
                in_ap=table_ap,
                idxs_ap=it_t[:, i0 + 8 * s0 : i0 + 8 * (s0 + sn)],
                num_idxs=128 * sn,
                num_idxs_reg=128 * sn,
                elem_size=table_ap.shape[-1],
                single_packet=SINGLE_PACKET,
                queue_num=qrr[0] % 4,
            )
            qrr[0] += 1

    with tile.TileContext(nc) as tc:
        with (
            tc.tile_pool(name="gpool", bufs=4) as gpool,
            tc.tile_pool(name="mpool", bufs=3) as mpool,
            tc.tile_pool(name="spool", bufs=6) as spool,
            tc.tile_pool(name="hpool", bufs=3) as hpool,
            tc.tile_pool(name="cpool", bufs=1) as cpool,
            tc.tile_pool(name="dpool", bufs=1, space="DRAM") as dpool,
            tc.tile_pool(name="psum", bufs=4, space="PSUM") as psum,
            tc.tile_pool(name="psumt", bufs=2, space="PSUM") as psumt,
            tc.tile_pool(name="psumy", bufs=2, space="PSUM") as psumy,
        ):
            ident = cpool.tile([128, 128], F16)
            make_identity(nc, ident[:])
            w2_t = cpool.tile([HID, OUT], F16)
            nc.sync.dma_start(out=w2_t[:], in_=w2[:])
            c2n_t = cpool.tile([OUT, 1], F32)
            nc.sync.dma_start(out=c2n_t[:], in_=c2n[:])
            pcw_t = cpool.tile([128, NW], F32)
            nc.sync.dma_start(out=pcw_t[:], in_=pcw[:])
            pcwB_t = cpool.tile([128, NW], F32)
            nc.sync.dma_start(out=pcwB_t[:], in_=pcwB[:])
            it_t = cpool.tile([128, ITOT], I16)
            nc.sync.dma_start(out=it_t[:], in_=idxs[:])
            esw_t = cpool.tile([128, TOTA], F16)
            nc.sync.dma_start(out=esw_t[:], in_=esw[:])
            eswB_t = cpool.tile([128, TOTB], F16)
            nc.sync.dma_start(out=eswB_t[:], in_=eswB[:])
            numB = dpool.tile([NPAD, NBF], F16)

            # ---- pass B: high-src partial sums into numB
            for g, (w0, w1_) in enumerate(groupsB):
                b0, b1 = int(offsB[w0]), int(offsB[w1_])
                nbg = b1 - b0
                gt = gpool.tile([128, nbg * HID], F16, tag="gt")
                gt3 = gt[:].rearrange("p (c f) -> p c f", f=HID)
                gather(nc, gt3, htab[HIBASE:], it_t, int(icoB[g]), nbg)
                tw = spool.tile([128, nbg], F32, tag="tw")
                nc.scalar.activation(
                    tw[:], eswB_t[:, b0:b1], AF.Tanh, scale=0.5
                )
                ex2 = spool.tile([128, nbg, 2], F16, tag="ex2")
                nc.scalar.activation(
                    ex2[:],
                    tw[:, :, None].to_broadcast([128, nbg, 2]),
                    AF.Exp,
                    scale=0.5,
                )
                gs = mpool.tile([128, nbg * HID], F16, tag="gs")
                nc.vector.tensor_tensor(
                    out=gs[:].rearrange("p (c a two) -> p c a two", a=64, two=2),
                    in0=gt3.rearrange("p c (a two) -> p c a two", two=2),
                    in1=ex2[:, :, None, :].to_broadcast([128, nbg, 64, 2]),
                    op=ALU.mult,
                )
                for w in range(w0, w1_):
                    ncB = int(nchB[w])
                    lo = int(offsB[w]) - b0
                    den = spool.tile([128, 1], F32, tag="den")
                    nc.vector.reduce_sum(
                        den[:], ex2[:, lo : lo + ncB, 0],
                        axis=mybir.AxisListType.X,
                    )
                    nc.vector.tensor_scalar(
                        out=den[:], in0=den[:], scalar1=pcwB_t[:, w : w + 1],
                        scalar2=None, op0=ALU.subtract,
                    )
                    acc = psum.tile([128, HID], F32, tag="acc")
                    for c in range(ncB):
                        nc.tensor.matmul(
                            out=acc[:],
                            lhsT=ident[:],
                            rhs=gs[:, (lo + c) * HID : (lo + c + 1) * HID],
                            start=(c == 0),
                            stop=(c == ncB - 1),
                        )
                    nB = hpool.tile([128, NBF], F16, tag="nB")
                    nc.scalar.activation(nB[:, :HID], acc[:], AF.Copy)
                    nc.vector.tensor_copy(nB[:, HID : HID + 1], den[:])
                    nc.sync.dma_start(
                        out=numB[:][w * 128 : (w + 1) * 128, :], in_=nB[:]
                    )

            # ---- main pass: low-src windows + numB merge
            for g, (w0, w1_) in enumerate(groupsA):
                a0, a1 = int(offsA[w0]), int(offsA[w1_])
                na = a1 - a0
                nwin = w1_ - w0
                i0 = int(icoA[g])
                gt = gpool.tile([128, na * HID], F16, tag="gt")
                gt3 = gt[:].rearrange("p (c f) -> p c f", f=HID)
                gather(nc, gt3, htab[:LOWROWS], it_t, i0, na)
                gBn = gpool.tile([128, nwin * NBF], F16, tag="gBn")
                gBn3 = gBn[:].rearrange("p (c f) -> p c f", f=NBF)
                nc.gpsimd.dma_gather(
                    out_ap=gBn3,
                    in_ap=numB[:],
                    idxs_ap=it_t[:, i0 + 8 * na : i0 + 8 * (na + nwin)],
                    num_idxs=128 * nwin,
                    num_idxs_reg=128 * nwin,
                    elem_size=NBF,
                    single_packet=SINGLE_PACKET,
                    queue_num=qrr[0] % 4,
                )
                qrr[0] += 1
                tw = spool.tile([128, na], F32, tag="tw")
                nc.scalar.activation(
                    tw[:], esw_t[:, a0:a1], AF.Tanh, scale=0.5
                )
                ex2 = spool.tile([128, na, 2], F16, tag="ex2")
                nc.scalar.activation(
                    ex2[:],
                    tw[:, :, None].to_broadcast([128, na, 2]),
                    AF.Exp,
                    scale=0.5,
                )
                gs = mpool.tile([128, na * HID], F16, tag="gs")
                nc.vector.tensor_tensor(
                    out=gs[:].rearrange("p (c a two) -> p c a two", a=64, two=2),
                    in0=gt3.rearrange("p c (a two) -> p c a two", two=2),
                    in1=ex2[:, :, None, :].to_broadcast([128, na, 64, 2]),
                    op=ALU.mult,
                )
                for w in range(w0, w1_):
                    ncA = int(nchA[w])
                    loA = int(offsA[w]) - a0
                    wl = w - w0
                    den = spool.tile([128, 1], F32, tag="den")
                    nc.vector.reduce_sum(
                        den[:], ex2[:, loA : loA + ncA, 0],
                        axis=mybir.AxisListType.X,
                    )
                    nc.vector.tensor_tensor(
                        out=den[:], in0=den[:],
                        in1=gBn3[:, wl, HID : HID + 1], op=ALU.add,
                    )
                    nc.vector.tensor_scalar(
                        out=den[:], in0=den[:], scalar1=pcw_t[:, w : w + 1],
                        scalar2=0.5, op0=ALU.subtract, op1=ALU.max,
                    )
                    recip = spool.tile([128, 1], F32, tag="recip")
                    nc.vector.reciprocal(recip[:], den[:])
                    acc = psum.tile([128, HID], F32, tag="acc")
                    for c in range(ncA):
                        nc.tensor.matmul(
                            out=acc[:],
                            lhsT=ident[:],
                            rhs=gs[:, (loA + c) * HID : (loA + c + 1) * HID],
                            start=(c == 0),
                            stop=False,
                        )
                    nc.tensor.matmul(
                        out=acc[:],
                        lhsT=ident[:],
                        rhs=gBn3[:, wl, :HID],
                        start=False,
                        stop=True,
                    )
                    # ELU+1: max(x,0) + exp(min(x,0)), x = acc*recip
                    mm = spool.tile([128, HID], F32, tag="mm")
                    nc.vector.tensor_scalar(
                        out=mm[:], in0=acc[:], scalar1=recip[:],
                        scalar2=0.0, op0=ALU.mult, op1=ALU.min,
                    )
                    rr = spool.tile([128, HID], F32, tag="rr")
                    nc.vector.tensor_scalar(
                        out=rr[:], in0=acc[:], scalar1=recip[:],
                        scalar2=0.0, op0=ALU.mult, op1=ALU.max,
                    )
                    ee = spool.tile([128, HID], F32, tag="ee")
                    nc.scalar.activation(ee[:], mm[:], AF.Exp)
                    h1 = hpool.tile([128, HID], F16, tag="h1")
                    nc.vector.tensor_tensor(
                        out=h1[:], in0=rr[:], in1=ee[:], op=ALU.add
                    )
                    # yT_w = W2^T @ h1^T - colsum(W2) (the ELU -1 term)
                    h1tp = psumt.tile([128, HID], F16, tag="h1tp")
                    nc.tensor.transpose(
                        out=h1tp[:], in_=h1[:], identity=ident[:]
                    )
                    h1t = hpool.tile([128, HID], F16, tag="h1t")
                    nc.scalar.activation(h1t[:], h1tp[:], AF.Copy)
                    yp = psumy.tile([OUT, 128], F32, tag="yp")
                    nc.tensor.matmul(
                        out=yp[:], lhsT=w2_t[:], rhs=h1t[:],
                        start=True, stop=True,
                    )
                    yt = hpool.tile([OUT, 128], F16, tag="yt")
                    nc.scalar.activation(
                        yt[:], yp[:], AF.Identity, bias=c2n_t[:]
                    )
                    nc.sync.dma_start(
                        out=yT[:, w * 128 : (w + 1) * 128], in_=yt[:]
                    )
    nc.finalize()
    return nc


# ---------------------------------------------------------------- host glue
def _wrap16(idx_cols):
    """[128, ncols] int16 slot indices -> dma_gather wrapped layout.

    Flattened order i = c*128 + p; idx i lives at (partition i%16,
    col i//16), replicated across the 8 groups of 16 partitions.
    Returns [128, 8*ncols]."""
    ncols = idx_cols.shape[1]
    flat = idx_cols.T.reshape(-1)                 # i = c*128 + p
    wrapped = flat.reshape(8 * ncols, 16).T       # [16, 8*ncols]
    return np.tile(wrapped, (8, 1)).astype(np.int16)


def _build_windows(dloc, nch_shared):
    """Degree-sort one core's nodes; per-window chunk counts."""
    order = np.argsort(-dloc, kind="stable")
    ds = dloc[order]
    for w in range(NW):
        j0 = w * 128
        if j0 < NPC:
            j1 = min(j0 + 128, NPC)
            nch_shared[w] = max(nch_shared[w], int(ds[j0:j1].max()))
    return order


def _fill_slots(order, degs, srcs_at, es_full, ed_full, core0, nch, offs, TOT,
                dummy_val, shift):
    """Per-core slot tables for one window structure.

    Returns (idx_arr int64 [128, TOT], esw fp16 [128, TOT],
    padcnt [128, NW])."""
    idx_arr = np.full((128, TOT), dummy_val, np.int64)
    esw_arr = np.full((128, TOT), -60000.0, np.float32)
    padcnt = np.zeros((128, NW), np.float32)
    for wloc in range(NW):
        j0 = wloc * 128
        nodes = order[j0 : j0 + 128]
        ncw = int(nch[wloc])
        for p, j in enumerate(nodes):
            g = core0 + j
            d = int(degs[g])
            ss = srcs_at(g, d)
            col = int(offs[wloc])
            idx_arr[p, col : col + d] = ss + shift
            esw_arr[p, col : col + d] = es_full[ss] + ed_full[g]
            esw_arr[p, col + d : col + ncw] += ed_full[g]
            padcnt[p, wloc] = ncw - d
        for p in range(len(nodes), 128):
            padcnt[p, wloc] = ncw
    return idx_arr, esw_arr, padcnt


def kernel(x, edge_index, W1, att_src, att_dst, W2):
    x = np.asarray(x, dtype=np.float32)
    edge_index = np.asarray(edge_index)
    W1 = np.asarray(W1, dtype=np.float32)
    att_src = np.asarray(att_src, dtype=np.float32)
    att_dst = np.asarray(att_dst, dtype=np.float32)
    W2 = np.asarray(W2, dtype=np.float32)

    src = edge_index[0].astype(np.int64)
    dst = edge_index[1].astype(np.int64)

    # ---- phase 1: sharded h/es/ed compute
    wcat = np.concatenate(
        [W1, (W1 @ att_src)[:, None], (W1 @ att_dst)[:, None]], axis=1
    ).astype(np.float16)
    xT = np.ascontiguousarray(x.T).astype(np.float16)  # [IN, N]

    nc1 = build_phase1()
    in_maps1 = []
    for c in range(NCORES):
        sh = xT[:, c * NPC : (c + 1) * NPC]
        if sh.shape[1] < NPAD:
            sh = np.concatenate(
                [sh, np.zeros((IN, NPAD - sh.shape[1]), np.float16)], axis=1
            )
        in_maps1.append({"xT": np.ascontiguousarray(sh), "wcat": wcat})
    trace = os.environ.get("BASS_GAT_TRACE") == "1"
    tkw = dict(trace=True, trace_cores=[0]) if trace else {}
    if trace:
        _patch_perfetto()
    t0 = time.time()
    res1 = run_bass_kernel_spmd(nc1, in_maps1, core_ids=list(range(NCORES)), **tkw)
    _timings["phase1_wall"] = time.time() - t0
    _timings["phase1_ns"] = res1.exec_time_ns

    htab = np.zeros((TABROWS, HID), np.float16)
    es_full = np.zeros(N, np.float32)
    ed_full = np.zeros(N, np.float32)
    for c in range(NCORES):
        hv = res1.results[c]["haug"][:NPC]
        htab[1 + c * NPC : 1 + (c + 1) * NPC] = hv[:, :HID]
        es_full[c * NPC : (c + 1) * NPC] = hv[:, HID].astype(np.float32)
        ed_full[c * NPC : (c + 1) * NPC] = hv[:, HID + 1].astype(np.float32)

    # ---- host edge routing
    deg = np.bincount(dst, minlength=N)
    is_low = (src + 1) < LOWROWS
    degA_full = np.bincount(dst[is_low], minlength=N)
    degB_full = deg - degA_full

    lkey = dst * 2 + (~is_low).astype(np.int64)   # low edges first per dst
    eorder = np.argsort(lkey, kind="stable")
    src_s = src[eorder]
    estarts = np.zeros(N + 1, np.int64)
    estarts[1:] = np.cumsum(deg)

    ordersA, ordersB = [], []
    nchA = np.zeros(NW, np.int64)
    nchB = np.zeros(NW, np.int64)
    for c in range(NCORES):
        sl = slice(c * NPC, (c + 1) * NPC)
        ordersA.append(_build_windows(degA_full[sl], nchA))
        ordersB.append(_build_windows(degB_full[sl], nchB))
    nchA = np.maximum(nchA, 1)
    nchB = np.maximum(nchB, 1)
    offsA = np.zeros(NW + 1, np.int64)
    offsA[1:] = np.cumsum(nchA)
    offsB = np.zeros(NW + 1, np.int64)
    offsB[1:] = np.cumsum(nchB)
    TOTA, TOTB = int(offsA[-1]), int(offsB[-1])

    def mkgroups(offs):
        groups = []
        w0 = 0
        while w0 < NW:
            w1_ = w0 + 1
            while w1_ < NW and offs[w1_ + 1] - offs[w0] <= GCOLS:
                w1_ += 1
            groups.append((w0, w1_))
            w0 = w1_
        return groups

    groupsA = mkgroups(offsA)
    groupsB = mkgroups(offsB)

    in_maps2 = []
    for c in range(NCORES):
        orderA, orderB = ordersA[c], ordersB[c]
        core0 = c * NPC

        def srcsA(g, d):
            s0 = estarts[g]
            return src_s[s0 : s0 + d]

        def srcsB(g, d):
            s0 = estarts[g] + int(degA_full[g])
            return src_s[s0 : s0 + d]

        idxA, eswA, padA = _fill_slots(
            orderA, degA_full, srcsA, es_full, ed_full, core0,
            nchA, offsA, TOTA, 0, 1,
        )
        idxB, eswB_arr, padB = _fill_slots(
            orderB, degB_full, srcsB, es_full, ed_full, core0,
            nchB, offsB, TOTB, DUMMY_HI_LOCAL, 1 - HIBASE,
        )
        # rank of node j (core-local) in the B window order
        rankB = np.empty(NPC, np.int64)
        rankB[orderB] = np.arange(NPC)

        iparts = []
        for (w0, w1_) in groupsB:
            iparts.append(
                _wrap16(idxB[:, offsB[w0] : offsB[w1_]].astype(np.int16))
            )
        for (w0, w1_) in groupsA:
            iparts.append(
                _wrap16(idxA[:, offsA[w0] : offsA[w1_]].astype(np.int16))
            )
            nb_idx = np.zeros((128, w1_ - w0), np.int64)
            for wloc in range(w0, w1_):
                nodes = orderA[wloc * 128 : wloc * 128 + 128]
                nb_idx[: len(nodes), wloc - w0] = rankB[nodes]
            iparts.append(_wrap16(nb_idx.astype(np.int16)))
        idxs_full = np.ascontiguousarray(np.concatenate(iparts, axis=1))
        in_maps2.append(
            {
                "htab": htab,
                "idxs": idxs_full,
                "esw": eswA.astype(np.float16),
                "eswB": eswB_arr.astype(np.float16),
                "pcw": (padA * np.float32(np.exp(-0.5))).astype(np.float32),
                "pcwB": (padB * np.float32(np.exp(-0.5))).astype(np.float32),
                "w2": W2.astype(np.float16),
                "c2n": -W2.sum(axis=0, dtype=np.float32)[:, None],
            }
        )

    nc2 = build_phase2(nchA, nchB, groupsA, groupsB)
    t0 = time.time()
    res2 = run_bass_kernel_spmd(nc2, in_maps2, core_ids=list(range(NCORES)), **tkw)
    _timings["phase2_wall"] = time.time() - t0
    _timings["phase2_ns"] = res2.exec_time_ns

    out = np.zeros((N, OUT), np.float32)
    for c in range(NCORES):
        yv = res2.results[c]["yT"].astype(np.float32).T  # [NPAD, OUT]
        order = ordersA[c]
        out[c * NPC + order] = yv[:NPC]
    return out
